# revision 33
# baseline (speedup 1.0000x reference)
"""AFNO layer Trainium2 kernel — data-parallel over the 16 (b,t) pairs, 2 per core.

Pipeline per (b,t), per core (all matmuls bf16, accumulate f32):
  LN1 (token-major, batched stats) -> fwd DFT to 288 kept modes (matmul vs
  precomputed cos/sin, output channel-major) -> block-diag complex mixing
  (packed 128x128 matmuls, gelu / softshrink epilogues) -> PE transpose ->
  inverse DFT (matmul, token-major) -> +h +x residual -> LN2 -> PE transpose
  -> MLP (768->3072 gelu ->768) -> +res2.

Host-side folds: ln1_g into w1 (per-block diag), ln1_b vanishes in kept modes
(kx=5..28 excludes 0), ln2_g/ln2_b into mw1/mb1. All constants are
host-transposed into single contiguous SBUF images (one DMA each, issued on
gpsimd so the sync queue serves the activations first).
"""

import numpy as np
import ml_dtypes

B, T, NX, NY, E, BS = 2, 8, 32, 32, 768, 64
NB = E // BS
YM = NY // 2 + 1
KM = 12
LAM = 0.01
MODES = 24 * KM          # 288 kept modes
NTOK = NX * NY           # 1024 tokens per (b,t)
BT_PER_CORE = 2
NCORES = 8
TOK_CORE = BT_PER_CORE * NTOK   # 2048
H4 = 4 * E               # 3072
EPS = 1e-5

_BF16 = ml_dtypes.bfloat16
_FP8 = ml_dtypes.float8_e4m3
S1 = 64.0   # host scale on mw1 (fp8 needs values ~1; folded out in gelu scale)
S2 = 64.0   # host scale on mw2 (folded out in the residual-add epilogue)

_CACHE = {}


def _install_trace_shim():
    """Best-effort: register the axon NTFF profiling hook so BASS_TRACE=1 works."""
    try:
        import types, sys
        if 'antenv.axon_hooks' in sys.modules:
            return
        import antenv  # noqa
        from trn_agent_boot.trn_boot import _ntff_profile_via_ctypes
        mod = types.ModuleType('antenv.axon_hooks')
        hook = _ntff_profile_via_ctypes('/opt/axon/libaxon_pjrt.so')
        mod.get_axon_ntff_profile_hook = lambda: hook
        mod.set_axon_ntff_profile_hook = lambda h: None
        sys.modules['antenv.axon_hooks'] = mod
        from concourse import bass_utils
        bass_utils.upload_artifacts = lambda tmpdir: tmpdir
    except Exception:
        pass


def _dft_matrices():
    """ArT (1024,288), AiT, BrT (288,1024), BiT as float32 (analytic, ortho norm)."""
    xx, yy = np.meshgrid(np.arange(NX), np.arange(NY), indexing='ij')
    sx = xx.ravel().astype(np.float64)
    sy = yy.ravel().astype(np.float64)
    kxs = np.arange(YM - KM, YM + KM, dtype=np.float64)   # 5..28
    kys = np.arange(KM, dtype=np.float64)                  # 0..11
    KX, KY = np.meshgrid(kxs, kys, indexing='ij')
    mkx = KX.ravel()
    mky = KY.ravel()
    ph = 2 * np.pi * (np.outer(sx, mkx) + np.outer(sy, mky)) / 32.0  # (1024,288)
    art = np.cos(ph) / 32.0
    ait = -np.sin(ph) / 32.0
    wk = np.where(mky == 0, 1.0, 2.0)
    brt = ((np.cos(ph) / 32.0) * wk).T.copy()   # (288,1024)
    bit = ((-np.sin(ph) / 32.0) * wk).T.copy()
    return (art.astype(np.float32), ait.astype(np.float32),
            brt.astype(np.float32), bit.astype(np.float32))


def _pack_blockdiag(w):
    """w: (NB,64,64) -> (6,128,128) pairs of blocks on the diagonal."""
    out = np.zeros((NB // 2, 2 * BS, 2 * BS), np.float32)
    for j in range(NB // 2):
        out[j, :BS, :BS] = w[2 * j]
        out[j, BS:, BS:] = w[2 * j + 1]
    return out


def _sb_image(a, p):
    """(n*p, f) -> (p, n*f): partition-major SBUF image for one big DMA."""
    n = a.shape[0] // p
    return np.ascontiguousarray(
        a.reshape(n, p, a.shape[1]).transpose(1, 0, 2).reshape(p, n * a.shape[1]))


def _build_program(ln1_trivial, mb2_zero, stage=5):
    import concourse.bass as bass
    import concourse.bacc as bacc
    import concourse.mybir as mybir
    from concourse import tile
    from concourse.tile import add_dep_helper

    f32 = mybir.dt.float32
    bf16 = mybir.dt.bfloat16
    fp8 = mybir.dt.float8e4
    DR = mybir.MatmulPerfMode.DoubleRow
    AF = mybir.ActivationFunctionType
    ALU = mybir.AluOpType
    AX = mybir.AxisListType

    nc = bacc.Bacc("TRN2", target_bir_lowering=False, debug=False)

    dp = nc.declare_dram_parameter
    x_d = dp("x", [TOK_CORE, E], bf16, isOutput=False)
    art_d = dp("art", [128, 8 * MODES], fp8, isOutput=False)
    ait_d = dp("ait", [128, 8 * MODES], fp8, isOutput=False)
    bcp_d = dp("bcp", [128, 4 * NTOK], fp8, isOutput=False)
    brem_d = dp("brem", [64, NTOK], bf16, isOutput=False)
    wmix_d = dp("wmix", [128, 6 * 128 * 6], bf16, isOutput=False)  # 6 packed mats
    bmix_d = dp("bmix", [128, 36], f32, isOutput=False)  # b1r b1i b2rm b2rn b2im b2in
    mw1_d = dp("mw1f", [128, 6 * H4], fp8, isOutput=False)
    mb1_d = dp("mb1f", [128, 24], f32, isOutput=False)
    mw2_d = dp("mw2", [128, 24 * E], fp8, isOutput=False)
    ident_d = dp("ident", [128, 128], bf16, isOutput=False)
    if not ln1_trivial:
        g1r_d = dp("g1rep", [128, E], f32, isOutput=False)
        b1lr_d = dp("b1rep", [128, E], f32, isOutput=False)
    if not mb2_zero:
        mb2r_d = dp("mb2rep", [128, E], f32, isOutput=False)
    out_d = dp("out", [TOK_CORE, E], bf16, isOutput=True)

    with tile.TileContext(nc) as tc:
        with (
            tc.tile_pool(name="pc", bufs=1) as pc,
            tc.tile_pool(name="p4", bufs=3) as p4,
            tc.tile_pool(name="p2", bufs=2) as p2,
            tc.tile_pool(name="p8", bufs=8) as p8,
            tc.tile_pool(name="pfq", bufs=16) as pfq,
            tc.tile_pool(name="po2", bufs=3) as po2,
            tc.tile_pool(name="phid", bufs=24) as phid,
            tc.tile_pool(name="px2", bufs=6) as px2,
            tc.tile_pool(name="pst", bufs=4) as pst,
            tc.tile_pool(name="psum", bufs=2, space="PSUM") as pp,
        ):
            # ---- constants: one consolidated DMA per tensor, on gpsimd ----
            art_all = pc.tile([128, 8 * MODES], fp8, tag="art")
            nc.gpsimd.dma_start(art_all[:], art_d[:])
            ait_all = pc.tile([128, 8 * MODES], fp8, tag="ait")
            nc.gpsimd.dma_start(ait_all[:], ait_d[:])
            wmix_all = pc.tile([128, 6 * 128 * 6], bf16, tag="wmix")
            nc.gpsimd.dma_start(wmix_all[:], wmix_d[:])
            bmix_all = pc.tile([128, 36], f32, tag="bmix")
            nc.gpsimd.dma_start(bmix_all[:], bmix_d[:])
            ident_t = pc.tile([128, 128], bf16, tag="ident")
            nc.gpsimd.dma_start(ident_t[:], ident_d[:])
            bcp_all = pc.tile([128, 4 * NTOK], fp8, tag="bcp")
            brem_all = pc.tile([64, NTOK], bf16, tag="brem")
            mw1_all = pc.tile([128, 6 * H4], fp8, tag="mw1")
            mw2_all = pc.tile([128, 24 * E], fp8, tag="mw2")
            mb1_all = pc.tile([128, 24], f32, tag="mb1")
            eps_t = pc.tile([128, 1], f32, tag="epsc")
            nc.vector.memset(eps_t[:], EPS)
            if not ln1_trivial:
                g1rep_t = pc.tile([128, E], f32, tag="g1rep")
                nc.gpsimd.dma_start(g1rep_t[:], g1r_d[:])
                b1rep_t = pc.tile([128, E], f32, tag="b1rep")
                nc.gpsimd.dma_start(b1rep_t[:], b1lr_d[:])
            if not mb2_zero:
                mb2rep_t = pc.tile([128, E], f32, tag="mb2rep")
                nc.gpsimd.dma_start(mb2rep_t[:], mb2r_d[:])

            artv = art_all[:].rearrange("p (q k m) -> p q k m", q=4, k=2)
            aitv = ait_all[:].rearrange("p (q k m) -> p q k m", q=4, k=2)
            bcpv = bcp_all[:].rearrange("p (c k t) -> p c k t", c=2, k=2)
            # wmix order: w1r w1i w1in w2r w2i w2in, each (128, 6*128)
            def wm(idx, j):
                o = idx * 6 * 128 + j * 128
                return wmix_all[:, o:o + 128]
            def bm(idx, j):
                return bmix_all[:, idx * 6 + j: idx * 6 + j + 1]
            mw1v = mw1_all[:].rearrange("p (q k f) -> p q k f", q=3, k=2)
            mw2v = mw2_all[:].rearrange("p (q k e) -> p q k e", q=12, k=2)
            def mw1_c(q, fj):
                return mw1v[:, q, :, fj * 128:(fj + 1) * 128]
            def mw2_c(q, n):
                return mw2v[:, q, :, n * 384:(n + 1) * 384]
            def mb1_c(fj):
                return mb1_all[:, fj:fj + 1]

            def layernorm(src_tiles, dst_pool, dst_tag, make_hx, bt=0, nameprefix=None,
                          pair_fp8=False):
                """LN over 8 (128,E) tiles via bn_stats -> normalized tiles.
                pair_fp8: write into 4 [128,2,E] fp8 pair tiles (DoubleRow layout).
                If make_hx, also emit hx = h + x tiles (bf16)."""
                outs = []
                hxs = []
                for i in range(8):
                    xt = src_tiles[i]
                    xr = xt[:].rearrange("p (n f) -> p n f", f=256)
                    stats = pst.tile([128, 3, 6], f32, tag="bst")
                    for s3 in range(3):
                        nc.vector.bn_stats(stats[:, s3, :], xr[:, s3, :])
                    mv = pst.tile([128, 2], f32, tag="mv")
                    nc.vector.bn_aggr(mv[:], stats[:])
                    std = pst.tile([128, 1], f32, tag="std")
                    nc.scalar.activation(std[:], mv[:, 1:2], AF.Sqrt, bias=eps_t[:])
                    rstd = pst.tile([128, 1], f32, tag="rstd")
                    nc.vector.reciprocal(rstd[:], std[:])
                    nmr = pst.tile([128, 1], f32, tag="nmr")
                    nc.vector.scalar_tensor_tensor(nmr[:], mv[:, 0:1], -1.0, rstd[:],
                                                   op0=ALU.mult, op1=ALU.mult)
                    if pair_fp8:
                        if i % 2 == 0:
                            hpt = dst_pool.tile([128, 2, E], fp8, tag=dst_tag,
                                                name=f"{nameprefix or dst_tag}_{bt}_{i // 2}")
                            outs.append(hpt)
                        dst_ap = outs[i // 2][:, i % 2, :]
                    else:
                        hb = dst_pool.tile([128, E], bf16, tag=dst_tag,
                                           name=f"{nameprefix or dst_tag}_{bt}_{i}")
                        outs.append(hb)
                        dst_ap = hb[:]
                    weng = nc.gpsimd if i % 2 == 0 else nc.vector
                    weng.tensor_scalar(dst_ap, xt[:], rstd[:], nmr[:],
                                       op0=ALU.mult, op1=ALU.add)
                    if make_hx:
                        hxt = p8.tile([128, E], bf16, tag="hx", name=f"hx_{bt}_{i}")
                        if ln1_trivial:
                            r1p = pst.tile([128, 1], f32, tag="r1p")
                            nc.vector.tensor_scalar_add(r1p[:], rstd[:], 1.0)
                            hxeng = nc.vector if i % 2 == 0 else nc.gpsimd
                            hxeng.tensor_scalar(hxt[:], xt[:], r1p[:], nmr[:],
                                                op0=ALU.mult, op1=ALU.add)
                        else:
                            tmp = p2.tile([128, E], f32, tag="lngtmp")
                            nc.vector.tensor_scalar(tmp[:], xt[:], rstd[:], nmr[:],
                                                    op0=ALU.mult, op1=ALU.add)
                            nc.vector.tensor_tensor(tmp[:], tmp[:], g1rep_t[:],
                                                    op=ALU.mult)
                            nc.vector.tensor_tensor(tmp[:], tmp[:], b1rep_t[:],
                                                    op=ALU.add)
                            nc.vector.tensor_tensor(hxt[:], tmp[:], xt[:],
                                                    op=ALU.add)
                        hxs.append(hxt)
                return outs, hxs

            # ---- per (b,t) pipeline ----
            for bt in range(BT_PER_CORE):
                base = bt * NTOK

                xts = []
                for i in range(8):
                    xt = p8.tile([128, E], bf16, tag="xin", name=f"x_{bt}_{i}", bufs=6)
                    eng = nc.sync if i % 2 == 0 else nc.scalar
                    eng.dma_start(xt[:], x_d[base + i * 128: base + (i + 1) * 128, :])
                    xts.append(xt)

                hbf, hx = layernorm(xts, p8, "hb", make_hx=True, bt=bt, pair_fp8=True)

                if stage <= 1:
                    for i in range(8):
                        ost = p4.tile([128, E], f32, tag="xio")
                        nc.vector.tensor_copy(ost[:], hx[i][:])
                        nc.sync.dma_start(out_d[base + i*128: base+(i+1)*128, :], ost[:])
                    continue

                # fwd DFT: FR/FI channel-major (e-chunk 128, 288), fp8 DoubleRow
                # over token pairs; psum holds 32*fr (art stored unscaled cos,
                # the /32 folded into w1 on host)
                frb = []
                fib = []
                for j in range(6):
                    pfr = pp.tile([128, MODES], f32, tag="mmA", bufs=2)
                    for q in range(4):
                        nc.tensor.matmul(pfr[:], hbf[q][:, :, j * 128:(j + 1) * 128],
                                         artv[:, q, :, :], start=(q == 0), stop=(q == 3),
                                         perf_mode=DR)
                    fr = pfq.tile([128, MODES], bf16, tag="fq", name=f"fr{j}")
                    nc.scalar.activation(fr[:], pfr[:], AF.Copy)
                    frb.append(fr)
                    pfi = pp.tile([128, MODES], f32, tag="mmA", bufs=2)
                    for q in range(4):
                        nc.tensor.matmul(pfi[:], hbf[q][:, :, j * 128:(j + 1) * 128],
                                         aitv[:, q, :, :], start=(q == 0), stop=(q == 3),
                                         perf_mode=DR)
                    fi = pfq.tile([128, MODES], bf16, tag="fq", name=f"fi{j}")
                    fi_copy = nc.scalar.activation(fi[:], pfi[:], AF.Copy)
                    fib.append(fi)

                if bt == 0:
                    # deferred weight loads: don't let these race the
                    # startup burst (x tiles + DFT matrices) on the HBM wire
                    d0 = nc.gpsimd.dma_start(bcp_all[:], bcp_d[:])
                    d0b = nc.gpsimd.dma_start(brem_all[:], brem_d[:])
                    d1 = nc.gpsimd.dma_start(mw1_all[:], mw1_d[:])
                    d2 = nc.gpsimd.dma_start(mw2_all[:], mw2_d[:])
                    d3 = nc.gpsimd.dma_start(mb1_all[:], mb1_d[:])
                    for dd in (d0, d0b, d1, d2, d3):
                        add_dep_helper(dd.ins, fi_copy.ins,
                                       reason="defer bulk weight DMA past fwd DFT")

                if stage <= 2:
                    for j in range(6):
                        ost = p4.tile([128, E], f32, tag="xio")
                        nc.vector.tensor_copy(ost[:, :MODES], frb[j][:])
                        nc.vector.tensor_copy(ost[:, MODES:2*MODES], fib[j][:])
                        nc.vector.memset(ost[:, 2*MODES:], 0.0)
                        nc.sync.dma_start(out_d[base + j*128: base+(j+1)*128, :], ost[:])
                    continue

                # mixing layer 1 (complex, gelu) and layer 2 (+softshrink);
                # transposes of the shrunk output fused per-j to keep PE fed
                if stage > 3:
                    # mode-major shrunk output for DoubleRow iDFT: two [128,2,E]
                    # fp8 pair tiles (slot0=real, slot1=imag; modes c*128..) plus
                    # a [64,E] bf16 remainder (modes 256:288, r stacked on i)
                    o2p = [po2.tile([128, 2, E], fp8, tag="o2p", name=f"o2p{c}")
                           for c in range(2)]
                    o2rem = po2.tile([64, E], bf16, tag="o2rem", name="o2rem")
                shr = []
                shi = []
                for j in range(6):
                    p1r = pp.tile([128, MODES], f32, tag="mmA", bufs=2)
                    nc.tensor.matmul(p1r[:], wm(0, j), frb[j][:], start=True, stop=False)
                    nc.tensor.matmul(p1r[:], wm(2, j), fib[j][:], start=False, stop=True)
                    o1r = pfq.tile([128, MODES], bf16, tag="fq", name=f"o1r{j}")
                    nc.scalar.activation(o1r[:], p1r[:], AF.Gelu, bias=bm(0, j))
                    p1i = pp.tile([128, MODES], f32, tag="mmA", bufs=2)
                    nc.tensor.matmul(p1i[:], wm(1, j), frb[j][:], start=True, stop=False)
                    nc.tensor.matmul(p1i[:], wm(0, j), fib[j][:], start=False, stop=True)
                    o1i = pfq.tile([128, MODES], bf16, tag="fq", name=f"o1i{j}")
                    nc.scalar.activation(o1i[:], p1i[:], AF.Gelu, bias=bm(1, j))

                    p2r = pp.tile([128, MODES], f32, tag="mmA", bufs=2)
                    nc.tensor.matmul(p2r[:], wm(3, j), o1r[:], start=True, stop=False)
                    nc.tensor.matmul(p2r[:], wm(5, j), o1i[:], start=False, stop=True)
                    t1 = p2.tile([128, MODES], bf16, tag="t1")
                    t2 = p2.tile([128, MODES], bf16, tag="t2")
                    nc.scalar.activation(t1[:], p2r[:], AF.Relu, bias=bm(2, j), scale=32.0)
                    nc.scalar.activation(t2[:], p2r[:], AF.Relu, bias=bm(3, j), scale=-32.0)
                    sr = pfq.tile([128, MODES], bf16, tag="fq", name=f"shr{j}")
                    nc.vector.tensor_sub(sr[:], t1[:], t2[:])
                    shr.append(sr)

                    p2i = pp.tile([128, MODES], f32, tag="mmA", bufs=2)
                    nc.tensor.matmul(p2i[:], wm(4, j), o1r[:], start=True, stop=False)
                    nc.tensor.matmul(p2i[:], wm(3, j), o1i[:], start=False, stop=True)
                    t3 = p2.tile([128, MODES], bf16, tag="t1")
                    t4 = p2.tile([128, MODES], bf16, tag="t2")
                    nc.scalar.activation(t3[:], p2i[:], AF.Relu, bias=bm(4, j), scale=32.0)
                    nc.scalar.activation(t4[:], p2i[:], AF.Relu, bias=bm(5, j), scale=-32.0)
                    si = pfq.tile([128, MODES], bf16, tag="fq", name=f"shi{j}")
                    nc.vector.tensor_sub(si[:], t3[:], t4[:])
                    shi.append(si)
                    if stage > 3:
                        for c in range(2):
                            ptr = pp.tile([128, 128], bf16, tag="tpm")
                            nc.tensor.transpose(ptr[:], sr[:, c * 128:(c + 1) * 128], ident_t[:])
                            nc.vector.tensor_copy(o2p[c][:, 0, j * 128:(j + 1) * 128], ptr[:])
                            pti = pp.tile([128, 128], bf16, tag="tpm")
                            nc.tensor.transpose(pti[:], si[:, c * 128:(c + 1) * 128], ident_t[:])
                            nc.vector.tensor_copy(o2p[c][:, 1, j * 128:(j + 1) * 128], pti[:])
                        ptr = pp.tile([128, 128], bf16, tag="tpm")
                        nc.tensor.transpose(ptr[0:32, :], sr[:, 256:288], ident_t[:])
                        nc.vector.tensor_copy(o2rem[0:32, j * 128:(j + 1) * 128], ptr[0:32, :])
                        pti = pp.tile([128, 128], bf16, tag="tpm")
                        nc.tensor.transpose(pti[0:32, :], si[:, 256:288], ident_t[:])
                        nc.vector.tensor_copy(o2rem[32:64, j * 128:(j + 1) * 128], pti[0:32, :])

                if stage <= 3:
                    for j in range(6):
                        ost = p4.tile([128, E], f32, tag="xio")
                        nc.vector.tensor_copy(ost[:, :MODES], shr[j][:])
                        nc.vector.tensor_copy(ost[:, MODES:2*MODES], shi[j][:])
                        nc.vector.memset(ost[:, 2*MODES:], 0.0)
                        nc.sync.dma_start(out_d[base + j*128: base+(j+1)*128, :], ost[:])
                    continue

                # inverse DFT + residual: out1 = spat + hx   (token-major, bf16)
                # psum = 1024*spat (32x in bcp/brem, 32x in the shrunk modes)
                out1 = []
                for p in range(8):
                    o1t = p8.tile([128, E], bf16, tag="out1", name=f"out1_{p}")
                    for n in range(2):
                        ps = pp.tile([128, 384], f32, tag="big", bufs=4)
                        for c in range(2):
                            nc.tensor.matmul(ps[:], bcpv[:, c, :, p * 128:(p + 1) * 128],
                                             o2p[c][:, :, n * 384:(n + 1) * 384],
                                             start=(c == 0), stop=False, perf_mode=DR)
                        nc.tensor.matmul(ps[:], brem_all[:, p * 128:(p + 1) * 128],
                                         o2rem[:, n * 384:(n + 1) * 384],
                                         start=False, stop=True)
                        nc.vector.scalar_tensor_tensor(
                            o1t[:, n * 384:(n + 1) * 384], ps[:], 1.0 / 1024.0,
                            hx[p][:, n * 384:(n + 1) * 384], op0=ALU.mult, op1=ALU.add)
                    out1.append(o1t)

                if stage <= 4:
                    for p in range(8):
                        ost = p4.tile([128, E], f32, tag="xio")
                        nc.vector.tensor_copy(ost[:], out1[p][:])
                        nc.sync.dma_start(out_d[base + p*128: base+(p+1)*128, :], ost[:])
                    continue

                # LN2 -> h2bf (normalized token-major bf16; affine folded into mw1/mb1)
                h2bf, _ = layernorm(out1, p8, "hb", make_hx=False, bt=bt, nameprefix="h2")

                # MLP in token-halves (transpose h2 -> channel-major per half).
                # fp8 DoubleRow matmuls: weights host-packed into k-pair images,
                # activations written into [128, 2, F] pair tiles.
                for h in range(2):
                    x2h = [px2.tile([128, 2, 512], fp8, tag="x2q",
                                    name=f"x2h{bt}_{h}_{q}") for q in range(3)]
                    for tcn in range(4):
                        p = h * 4 + tcn
                        for j in range(6):
                            pt = pp.tile([128, 128], bf16, tag="tpm")
                            nc.tensor.transpose(pt[:], h2bf[p][:, j * 128:(j + 1) * 128],
                                                ident_t[:])
                            nc.vector.tensor_copy(
                                x2h[j // 2][:, j % 2, tcn * 128:(tcn + 1) * 128], pt[:])
                    hid = []
                    for qq in range(12):
                        hq = phid.tile([128, 2, 512], fp8, tag="hid",
                                       name=f"hid{bt}_{h}_{qq}")
                        hid.append(hq)
                    for fj in range(24):
                        ph = pp.tile([128, 512], f32, tag="big", bufs=4)
                        for q in range(3):
                            nc.tensor.matmul(ph[:], mw1_c(q, fj), x2h[q][:],
                                             start=(q == 0), stop=(q == 2),
                                             perf_mode=DR)
                        nc.scalar.activation(hid[fj // 2][:, fj % 2, :], ph[:],
                                             AF.Gelu, bias=mb1_c(fj), scale=1.0 / S1)
                    for tcn in range(4):
                        p = h * 4 + tcn
                        ost = p8.tile([128, E], bf16, tag="xin", name=f"ost{bt}_{h}_{tcn}", bufs=6)
                        for n in range(2):
                            po = pp.tile([128, 384], f32, tag="big", bufs=4)
                            for qq in range(12):
                                nc.tensor.matmul(po[:],
                                                 hid[qq][:, :, tcn * 128:(tcn + 1) * 128],
                                                 mw2_c(qq, n),
                                                 start=(qq == 0), stop=(qq == 11),
                                                 perf_mode=DR)
                            nc.vector.scalar_tensor_tensor(
                                ost[:, n * 384:(n + 1) * 384], po[:], 1.0 / S2,
                                out1[p][:, n * 384:(n + 1) * 384],
                                op0=ALU.mult, op1=ALU.add)
                        if not mb2_zero:
                            nc.vector.tensor_add(ost[:], ost[:], mb2rep_t[:])
                        oeng = nc.sync if p % 2 == 0 else nc.scalar
                        oeng.dma_start(
                            out_d[base + p * 128: base + (p + 1) * 128, :], ost[:])

    nc.compile()
    return nc


LAST_EXEC_NS = None


def make_consts(w1, b1, w2, b2, ln1_g, ln1_b, ln2_g, ln2_b,
                mw1, mb1, mw2, mb2, ln1_trivial, mb2_zero):
    art, ait, brt, bit = _dft_matrices()

    # fold ln1_g into w1 (left-diag per block over the i axis)
    g_blocks = ln1_g.reshape(NB, BS)
    W1R = _pack_blockdiag(w1[0] * g_blocks[:, :, None])
    W1I = _pack_blockdiag(w1[1] * g_blocks[:, :, None])
    W2R = _pack_blockdiag(w2[0])
    W2I = _pack_blockdiag(w2[1])

    b1r = b1[0].reshape(E)
    b1i = b1[1].reshape(E)
    b2r = b2[0].reshape(E)
    b2i = b2[1].reshape(E)

    mw1f = mw1 * ln2_g[:, None]
    mb1f = (mb1 + ln2_b @ mw1).reshape(H4)

    def bf(a):
        return np.ascontiguousarray(a.astype(_BF16))

    def fp8_pairs(a, scale):
        """(2q*128, F) -> (128, q*2*F) k-pair-interleaved fp8 image for DoubleRow."""
        nq = a.shape[0] // 256
        img = (a * scale).reshape(nq, 2, 128, a.shape[1]) \
            .transpose(2, 0, 1, 3).reshape(128, 2 * nq * a.shape[1])
        return np.ascontiguousarray(img.astype(_FP8))

    # wmix image: (128, 6 mats * 6 blocks * 128), order w1r w1i w1in w2r w2i w2in
    # w1 carries the 1/32 that was removed from the fp8 DFT matrices
    mats = [W1R / 32.0, W1I / 32.0, -W1I / 32.0, W2R, W2I, -W2I]
    wmix = np.concatenate(
        [m.transpose(1, 0, 2).reshape(128, 6 * 128) for m in mats], axis=1)
    # bmix image: (128, 36): 6 vectors x 6 chunks; shrink biases carry the
    # 32x fp8-friendly scale on the shrunk modes (undone by 1/1024 after iDFT)
    bvecs = [b1r, b1i, 32.0 * (b2r - LAM), 32.0 * (-b2r - LAM),
             32.0 * (b2i - LAM), 32.0 * (-b2i - LAM)]
    bmix = np.concatenate([v.reshape(6, 128).T for v in bvecs], axis=1)

    # iDFT pair image: [128, c(2), k(2), 1024] slot0=brt, slot1=bit (32x scale)
    brt32 = brt * 32.0
    bit32 = bit * 32.0
    bcp = np.zeros((128, 2, 2, NTOK), np.float32)
    for c in range(2):
        bcp[:, c, 0, :] = brt32[c * 128:(c + 1) * 128]
        bcp[:, c, 1, :] = bit32[c * 128:(c + 1) * 128]
    brem = np.concatenate([brt32[256:288], bit32[256:288]], axis=0)

    consts = {
        "art": fp8_pairs(art, 32.0), "ait": fp8_pairs(ait, 32.0),
        "bcp": np.ascontiguousarray(bcp.reshape(128, 4 * NTOK).astype(_FP8)),
        "brem": bf(brem),
        "wmix": bf(wmix), "bmix": np.ascontiguousarray(bmix, np.float32),
        "mw1f": fp8_pairs(mw1f, S1),
        "mb1f": np.ascontiguousarray(mb1f.reshape(24, 128).T, np.float32),
        "mw2": fp8_pairs(mw2, S2),
        "ident": bf(np.eye(128, dtype=np.float32)),
    }
    if not ln1_trivial:
        consts["g1rep"] = np.tile(ln1_g[None, :], (128, 1)).astype(np.float32)
        consts["b1rep"] = np.tile(ln1_b[None, :], (128, 1)).astype(np.float32)
    if not mb2_zero:
        consts["mb2rep"] = np.tile(mb2[None, :], (128, 1)).astype(np.float32)
    return consts


def kernel(input, w1, b1, w2, b2, ln1_g, ln1_b, ln2_g, ln2_b, mw1, mb1, mw2, mb2):
    global LAST_EXEC_NS
    _install_trace_shim()
    import os
    from concourse.bass_utils import run_bass_kernel_spmd

    input = np.asarray(input, np.float32)
    w1 = np.asarray(w1, np.float32)
    b1 = np.asarray(b1, np.float32)
    w2 = np.asarray(w2, np.float32)
    b2 = np.asarray(b2, np.float32)
    ln1_g = np.asarray(ln1_g, np.float32)
    ln1_b = np.asarray(ln1_b, np.float32)
    ln2_g = np.asarray(ln2_g, np.float32)
    ln2_b = np.asarray(ln2_b, np.float32)
    mw1 = np.asarray(mw1, np.float32)
    mb1 = np.asarray(mb1, np.float32)
    mw2 = np.asarray(mw2, np.float32)
    mb2 = np.asarray(mb2, np.float32)

    ln1_trivial = bool(np.all(ln1_g == 1.0) and np.all(ln1_b == 0.0))
    mb2_zero = bool(np.all(mb2 == 0.0))

    key = (ln1_trivial, mb2_zero)
    if key not in _CACHE:
        _CACHE[key] = _build_program(ln1_trivial, mb2_zero)
    nc = _CACHE[key]

    consts = make_consts(w1, b1, w2, b2, ln1_g, ln1_b, ln2_g, ln2_b,
                         mw1, mb1, mw2, mb2, ln1_trivial, mb2_zero)

    xs = input.reshape(B * T, NTOK, E)
    in_maps = []
    for c in range(NCORES):
        shard = np.ascontiguousarray(
            xs[c * BT_PER_CORE:(c + 1) * BT_PER_CORE].reshape(TOK_CORE, E)
            .astype(_BF16))
        m = {"x": shard}
        m.update(consts)
        in_maps.append(m)

    trace = bool(os.environ.get("BASS_TRACE"))
    res = run_bass_kernel_spmd(nc, in_maps, core_ids=list(range(NCORES)),
                               trace=trace)
    LAST_EXEC_NS = res.exec_time_ns
    out = np.concatenate(
        [np.asarray(res.results[c]["out"]).astype(np.float32)
         .reshape(BT_PER_CORE, NTOK, E) for c in range(NCORES)], axis=0)
    return out.reshape(B, T, NTOK, E)



# revision 37
# speedup vs baseline: 1.0719x; 1.0719x over previous
"""AFNO layer Trainium2 kernel — data-parallel over the 16 (b,t) pairs, 2 per core.

Pipeline per (b,t), per core (all matmuls bf16, accumulate f32):
  LN1 (token-major, batched stats) -> fwd DFT to 288 kept modes (matmul vs
  precomputed cos/sin, output channel-major) -> block-diag complex mixing
  (packed 128x128 matmuls, gelu / softshrink epilogues) -> PE transpose ->
  inverse DFT (matmul, token-major) -> +h +x residual -> LN2 -> PE transpose
  -> MLP (768->3072 gelu ->768) -> +res2.

Host-side folds: ln1_g into w1 (per-block diag), ln1_b vanishes in kept modes
(kx=5..28 excludes 0), ln2_g/ln2_b into mw1/mb1. All constants are
host-transposed into single contiguous SBUF images (one DMA each, issued on
gpsimd so the sync queue serves the activations first).
"""

import numpy as np
import ml_dtypes

B, T, NX, NY, E, BS = 2, 8, 32, 32, 768, 64
NB = E // BS
YM = NY // 2 + 1
KM = 12
LAM = 0.01
MODES = 24 * KM          # 288 kept modes
NTOK = NX * NY           # 1024 tokens per (b,t)
BT_PER_CORE = 2
NCORES = 8
TOK_CORE = BT_PER_CORE * NTOK   # 2048
H4 = 4 * E               # 3072
EPS = 1e-5

_BF16 = ml_dtypes.bfloat16
_FP8 = ml_dtypes.float8_e4m3
S1 = 64.0   # host scale on mw1 (fp8 needs values ~1; folded out in gelu scale)
S2 = 64.0   # host scale on mw2 (folded out in the residual-add epilogue)

_CACHE = {}


def _install_trace_shim():
    """Best-effort: register the axon NTFF profiling hook so BASS_TRACE=1 works."""
    try:
        import types, sys
        if 'antenv.axon_hooks' in sys.modules:
            return
        import antenv  # noqa
        from trn_agent_boot.trn_boot import _ntff_profile_via_ctypes
        mod = types.ModuleType('antenv.axon_hooks')
        hook = _ntff_profile_via_ctypes('/opt/axon/libaxon_pjrt.so')
        mod.get_axon_ntff_profile_hook = lambda: hook
        mod.set_axon_ntff_profile_hook = lambda h: None
        sys.modules['antenv.axon_hooks'] = mod
        from concourse import bass_utils
        bass_utils.upload_artifacts = lambda tmpdir: tmpdir
    except Exception:
        pass


def _dft_matrices():
    """ArT (1024,288), AiT, BrT (288,1024), BiT as float32 (analytic, ortho norm)."""
    xx, yy = np.meshgrid(np.arange(NX), np.arange(NY), indexing='ij')
    sx = xx.ravel().astype(np.float64)
    sy = yy.ravel().astype(np.float64)
    kxs = np.arange(YM - KM, YM + KM, dtype=np.float64)   # 5..28
    kys = np.arange(KM, dtype=np.float64)                  # 0..11
    KX, KY = np.meshgrid(kxs, kys, indexing='ij')
    mkx = KX.ravel()
    mky = KY.ravel()
    ph = 2 * np.pi * (np.outer(sx, mkx) + np.outer(sy, mky)) / 32.0  # (1024,288)
    art = np.cos(ph) / 32.0
    ait = -np.sin(ph) / 32.0
    wk = np.where(mky == 0, 1.0, 2.0)
    brt = ((np.cos(ph) / 32.0) * wk).T.copy()   # (288,1024)
    bit = ((-np.sin(ph) / 32.0) * wk).T.copy()
    return (art.astype(np.float32), ait.astype(np.float32),
            brt.astype(np.float32), bit.astype(np.float32))


def _pack_blockdiag(w):
    """w: (NB,64,64) -> (6,128,128) pairs of blocks on the diagonal."""
    out = np.zeros((NB // 2, 2 * BS, 2 * BS), np.float32)
    for j in range(NB // 2):
        out[j, :BS, :BS] = w[2 * j]
        out[j, BS:, BS:] = w[2 * j + 1]
    return out


def _sb_image(a, p):
    """(n*p, f) -> (p, n*f): partition-major SBUF image for one big DMA."""
    n = a.shape[0] // p
    return np.ascontiguousarray(
        a.reshape(n, p, a.shape[1]).transpose(1, 0, 2).reshape(p, n * a.shape[1]))


def _build_program(ln1_trivial, mb2_zero, stage=5):
    import concourse.bass as bass
    import concourse.bacc as bacc
    import concourse.mybir as mybir
    from concourse import tile
    from concourse.tile import add_dep_helper

    f32 = mybir.dt.float32
    bf16 = mybir.dt.bfloat16
    fp8 = mybir.dt.float8e4
    DR = mybir.MatmulPerfMode.DoubleRow
    AF = mybir.ActivationFunctionType
    ALU = mybir.AluOpType
    AX = mybir.AxisListType

    nc = bacc.Bacc("TRN2", target_bir_lowering=False, debug=False)

    dp = nc.declare_dram_parameter
    x_d = dp("x", [TOK_CORE, E], bf16, isOutput=False)
    art_d = dp("art", [128, 8 * MODES], fp8, isOutput=False)
    ait_d = dp("ait", [128, 8 * MODES], fp8, isOutput=False)
    bcp_d = dp("bcp", [128, 4 * NTOK], fp8, isOutput=False)
    brem_d = dp("brem", [64, NTOK], bf16, isOutput=False)
    wmix_d = dp("wmix", [128, 6 * 128 * 6], bf16, isOutput=False)  # 6 packed mats
    bmix_d = dp("bmix", [128, 36], f32, isOutput=False)  # b1r b1i b2rm b2rn b2im b2in
    mw1_d = dp("mw1f", [128, 6 * H4], fp8, isOutput=False)
    mb1_d = dp("mb1f", [128, 24], f32, isOutput=False)
    mw2_d = dp("mw2", [128, 24 * E], fp8, isOutput=False)
    ident_d = dp("ident", [128, 128], bf16, isOutput=False)
    if not ln1_trivial:
        g1r_d = dp("g1rep", [128, E], f32, isOutput=False)
        b1lr_d = dp("b1rep", [128, E], f32, isOutput=False)
    if not mb2_zero:
        mb2r_d = dp("mb2rep", [128, E], f32, isOutput=False)
    out_d = dp("out", [TOK_CORE, E], bf16, isOutput=True)

    with tile.TileContext(nc) as tc:
        with (
            tc.tile_pool(name="pc", bufs=1) as pc,
            tc.tile_pool(name="p4", bufs=3) as p4,
            tc.tile_pool(name="p2", bufs=2) as p2,
            tc.tile_pool(name="p8", bufs=8) as p8,
            tc.tile_pool(name="pfq", bufs=28) as pfq,
            tc.tile_pool(name="po2", bufs=3) as po2,
            tc.tile_pool(name="phid", bufs=24) as phid,
            tc.tile_pool(name="px2", bufs=6) as px2,
            tc.tile_pool(name="pst", bufs=4) as pst,
            tc.tile_pool(name="psum", bufs=2, space="PSUM") as pp,
        ):
            # ---- constants: one consolidated DMA per tensor, on gpsimd ----
            art_all = pc.tile([128, 8 * MODES], fp8, tag="art")
            nc.gpsimd.dma_start(art_all[:], art_d[:])
            ait_all = pc.tile([128, 8 * MODES], fp8, tag="ait")
            nc.gpsimd.dma_start(ait_all[:], ait_d[:])
            wmix_all = pc.tile([128, 6 * 128 * 6], bf16, tag="wmix")
            nc.gpsimd.dma_start(wmix_all[:], wmix_d[:])
            bmix_all = pc.tile([128, 36], f32, tag="bmix")
            nc.gpsimd.dma_start(bmix_all[:], bmix_d[:])
            ident_t = pc.tile([128, 128], bf16, tag="ident")
            nc.gpsimd.dma_start(ident_t[:], ident_d[:])
            bcp_all = pc.tile([128, 4 * NTOK], fp8, tag="bcp")
            brem_all = pc.tile([64, NTOK], bf16, tag="brem")
            mw1_all = pc.tile([128, 6 * H4], fp8, tag="mw1")
            mw2_all = pc.tile([128, 24 * E], fp8, tag="mw2")
            mb1_all = pc.tile([128, 24], f32, tag="mb1")
            eps_t = pc.tile([128, 1], f32, tag="epsc")
            nc.vector.memset(eps_t[:], EPS)
            if not ln1_trivial:
                g1rep_t = pc.tile([128, E], f32, tag="g1rep")
                nc.gpsimd.dma_start(g1rep_t[:], g1r_d[:])
                b1rep_t = pc.tile([128, E], f32, tag="b1rep")
                nc.gpsimd.dma_start(b1rep_t[:], b1lr_d[:])
            if not mb2_zero:
                mb2rep_t = pc.tile([128, E], f32, tag="mb2rep")
                nc.gpsimd.dma_start(mb2rep_t[:], mb2r_d[:])

            artv = art_all[:].rearrange("p (q k m) -> p q k m", q=4, k=2)
            aitv = ait_all[:].rearrange("p (q k m) -> p q k m", q=4, k=2)
            bcpv = bcp_all[:].rearrange("p (c k t) -> p c k t", c=2, k=2)
            # wmix order: w1r w1i w1in w2r w2i w2in, each (128, 6*128)
            def wm(idx, j):
                o = idx * 6 * 128 + j * 128
                return wmix_all[:, o:o + 128]
            def bm(idx, j):
                return bmix_all[:, idx * 6 + j: idx * 6 + j + 1]
            mw1v = mw1_all[:].rearrange("p (q k f) -> p q k f", q=3, k=2)
            mw2v = mw2_all[:].rearrange("p (q k e) -> p q k e", q=12, k=2)
            def mw1_c(q, fj):
                return mw1v[:, q, :, fj * 128:(fj + 1) * 128]
            def mw2_c(q, n):
                return mw2v[:, q, :, n * 384:(n + 1) * 384]
            def mb1_c(fj):
                return mb1_all[:, fj:fj + 1]

            def layernorm(src_tiles, dst_pool, dst_tag, make_hx, bt=0, nameprefix=None,
                          pair_fp8=False):
                """LN over 8 (128,E) tiles via bn_stats -> normalized tiles.
                pair_fp8: write into 4 [128,2,E] fp8 pair tiles (DoubleRow layout).
                If make_hx, also emit hx = h + x tiles (bf16)."""
                outs = []
                hxs = []
                for i in range(8):
                    xt = src_tiles[i]
                    xr = xt[:].rearrange("p (n f) -> p n f", f=256)
                    stats = pst.tile([128, 3, 6], f32, tag="bst")
                    for s3 in range(3):
                        nc.vector.bn_stats(stats[:, s3, :], xr[:, s3, :])
                    mv = pst.tile([128, 2], f32, tag="mv")
                    nc.vector.bn_aggr(mv[:], stats[:])
                    std = pst.tile([128, 1], f32, tag="std")
                    nc.scalar.activation(std[:], mv[:, 1:2], AF.Sqrt, bias=eps_t[:])
                    rstd = pst.tile([128, 1], f32, tag="rstd")
                    nc.vector.reciprocal(rstd[:], std[:])
                    nmr = pst.tile([128, 1], f32, tag="nmr")
                    nc.vector.scalar_tensor_tensor(nmr[:], mv[:, 0:1], -1.0, rstd[:],
                                                   op0=ALU.mult, op1=ALU.mult)
                    if pair_fp8:
                        if i % 2 == 0:
                            hpt = dst_pool.tile([128, 2, E], fp8, tag=dst_tag, bufs=8,
                                                name=f"{nameprefix or dst_tag}_{bt}_{i // 2}")
                            outs.append(hpt)
                        dst_ap = outs[i // 2][:, i % 2, :]
                    else:
                        hb = dst_pool.tile([128, E], bf16, tag=dst_tag, bufs=16,
                                           name=f"{nameprefix or dst_tag}_{bt}_{i}")
                        outs.append(hb)
                        dst_ap = hb[:]
                    weng = nc.gpsimd if i % 2 == 0 else nc.vector
                    weng.tensor_scalar(dst_ap, xt[:], rstd[:], nmr[:],
                                       op0=ALU.mult, op1=ALU.add)
                    if make_hx:
                        hxt = p8.tile([128, E], bf16, tag="hx", bufs=16,
                                      name=f"hx_{bt}_{i}")
                        if ln1_trivial:
                            r1p = pst.tile([128, 1], f32, tag="r1p")
                            nc.vector.tensor_scalar_add(r1p[:], rstd[:], 1.0)
                            hxeng = nc.vector if i % 2 == 0 else nc.gpsimd
                            hxeng.tensor_scalar(hxt[:], xt[:], r1p[:], nmr[:],
                                                op0=ALU.mult, op1=ALU.add)
                        else:
                            tmp = p2.tile([128, E], f32, tag="lngtmp")
                            nc.vector.tensor_scalar(tmp[:], xt[:], rstd[:], nmr[:],
                                                    op0=ALU.mult, op1=ALU.add)
                            nc.vector.tensor_tensor(tmp[:], tmp[:], g1rep_t[:],
                                                    op=ALU.mult)
                            nc.vector.tensor_tensor(tmp[:], tmp[:], b1rep_t[:],
                                                    op=ALU.add)
                            nc.vector.tensor_tensor(hxt[:], tmp[:], xt[:],
                                                    op=ALU.add)
                        hxs.append(hxt)
                return outs, hxs

            # ---- phase-interleaved pipeline over the two (b,t) shards: issue
            # order A0 A1 B0 B1 C0 C1 D0 E0 D1 E1 F00 F10 F01 F11 so one
            # shard's matmuls cover the other's LN/epilogue latency bubbles
            st = [dict() for _ in range(BT_PER_CORE)]

            def phase_A(bt):
                base = bt * NTOK
                xts = []
                for i in range(8):
                    xt = p8.tile([128, E], bf16, tag="xin", name=f"x_{bt}_{i}", bufs=16)
                    eng = nc.sync if i % 2 == 0 else nc.scalar
                    eng.dma_start(xt[:], x_d[base + i * 128: base + (i + 1) * 128, :])
                    xts.append(xt)
                hbf, hx = layernorm(xts, p8, "hb", make_hx=True, bt=bt, pair_fp8=True)
                st[bt]['hbf'] = hbf
                st[bt]['hx'] = hx

            def phase_B(bt):
                # fwd DFT: FR/FI channel-major (e-chunk 128, 288), fp8 DoubleRow
                # over token pairs; psum holds 32*fr (art stored unscaled cos,
                # the /32 folded into w1 on host)
                hbf = st[bt]['hbf']
                frb = []
                fib = []
                last = None
                for j in range(6):
                    pfr = pp.tile([128, MODES], f32, tag="mmA", bufs=2)
                    for q in range(4):
                        nc.tensor.matmul(pfr[:], hbf[q][:, :, j * 128:(j + 1) * 128],
                                         artv[:, q, :, :], start=(q == 0), stop=(q == 3),
                                         perf_mode=DR)
                    fr = pfq.tile([128, MODES], bf16, tag="fq", name=f"fr{bt}_{j}")
                    nc.scalar.activation(fr[:], pfr[:], AF.Copy)
                    frb.append(fr)
                    pfi = pp.tile([128, MODES], f32, tag="mmA", bufs=2)
                    for q in range(4):
                        nc.tensor.matmul(pfi[:], hbf[q][:, :, j * 128:(j + 1) * 128],
                                         aitv[:, q, :, :], start=(q == 0), stop=(q == 3),
                                         perf_mode=DR)
                    fi = pfq.tile([128, MODES], bf16, tag="fq", name=f"fi{bt}_{j}")
                    last = nc.scalar.activation(fi[:], pfi[:], AF.Copy)
                    fib.append(fi)
                st[bt]['frb'] = frb
                st[bt]['fib'] = fib
                return last

            def phase_C(bt):
                # mixing layer 1 (complex, gelu) and layer 2 (+softshrink);
                # shrunk output transposed per-j into the DoubleRow iDFT layout:
                # two [128,2,E] fp8 pair tiles (slot0=real slot1=imag) + [64,E]
                # bf16 remainder (modes 256:288, r stacked on i)
                frb = st[bt]['frb']
                fib = st[bt]['fib']
                o2p = [po2.tile([128, 2, E], fp8, tag="o2p", bufs=4,
                                name=f"o2p{bt}_{c}") for c in range(2)]
                o2rem = po2.tile([64, E], bf16, tag="o2rem", bufs=2,
                                 name=f"o2rem{bt}")
                for j in range(6):
                    p1r = pp.tile([128, MODES], f32, tag="mmA", bufs=2)
                    nc.tensor.matmul(p1r[:], wm(0, j), frb[j][:], start=True, stop=False)
                    nc.tensor.matmul(p1r[:], wm(2, j), fib[j][:], start=False, stop=True)
                    o1r = pfq.tile([128, MODES], bf16, tag="fq", name=f"o1r{bt}_{j}")
                    nc.scalar.activation(o1r[:], p1r[:], AF.Gelu, bias=bm(0, j))
                    p1i = pp.tile([128, MODES], f32, tag="mmA", bufs=2)
                    nc.tensor.matmul(p1i[:], wm(1, j), frb[j][:], start=True, stop=False)
                    nc.tensor.matmul(p1i[:], wm(0, j), fib[j][:], start=False, stop=True)
                    o1i = pfq.tile([128, MODES], bf16, tag="fq", name=f"o1i{bt}_{j}")
                    nc.scalar.activation(o1i[:], p1i[:], AF.Gelu, bias=bm(1, j))

                    p2r = pp.tile([128, MODES], f32, tag="mmA", bufs=2)
                    nc.tensor.matmul(p2r[:], wm(3, j), o1r[:], start=True, stop=False)
                    nc.tensor.matmul(p2r[:], wm(5, j), o1i[:], start=False, stop=True)
                    t1 = p2.tile([128, MODES], bf16, tag="t1")
                    t2 = p2.tile([128, MODES], bf16, tag="t2")
                    nc.scalar.activation(t1[:], p2r[:], AF.Relu, bias=bm(2, j), scale=32.0)
                    nc.scalar.activation(t2[:], p2r[:], AF.Relu, bias=bm(3, j), scale=-32.0)
                    sr = pfq.tile([128, MODES], bf16, tag="fq", name=f"shr{bt}_{j}")
                    nc.vector.tensor_sub(sr[:], t1[:], t2[:])

                    p2i = pp.tile([128, MODES], f32, tag="mmA", bufs=2)
                    nc.tensor.matmul(p2i[:], wm(4, j), o1r[:], start=True, stop=False)
                    nc.tensor.matmul(p2i[:], wm(3, j), o1i[:], start=False, stop=True)
                    t3 = p2.tile([128, MODES], bf16, tag="t1")
                    t4 = p2.tile([128, MODES], bf16, tag="t2")
                    nc.scalar.activation(t3[:], p2i[:], AF.Relu, bias=bm(4, j), scale=32.0)
                    nc.scalar.activation(t4[:], p2i[:], AF.Relu, bias=bm(5, j), scale=-32.0)
                    si = pfq.tile([128, MODES], bf16, tag="fq", name=f"shi{bt}_{j}")
                    nc.vector.tensor_sub(si[:], t3[:], t4[:])

                    for c in range(2):
                        ptr = pp.tile([128, 128], bf16, tag="tpm")
                        nc.tensor.transpose(ptr[:], sr[:, c * 128:(c + 1) * 128], ident_t[:])
                        nc.vector.tensor_copy(o2p[c][:, 0, j * 128:(j + 1) * 128], ptr[:])
                        pti = pp.tile([128, 128], bf16, tag="tpm")
                        nc.tensor.transpose(pti[:], si[:, c * 128:(c + 1) * 128], ident_t[:])
                        nc.vector.tensor_copy(o2p[c][:, 1, j * 128:(j + 1) * 128], pti[:])
                    ptr = pp.tile([128, 128], bf16, tag="tpm")
                    nc.tensor.transpose(ptr[0:32, :], sr[:, 256:288], ident_t[:])
                    nc.vector.tensor_copy(o2rem[0:32, j * 128:(j + 1) * 128], ptr[0:32, :])
                    pti = pp.tile([128, 128], bf16, tag="tpm")
                    nc.tensor.transpose(pti[0:32, :], si[:, 256:288], ident_t[:])
                    nc.vector.tensor_copy(o2rem[32:64, j * 128:(j + 1) * 128], pti[0:32, :])
                st[bt]['o2p'] = o2p
                st[bt]['o2rem'] = o2rem

            def phase_D(bt):
                # inverse DFT + residual, in place: out1 = hx += spat
                # psum = 1024*spat (32x in bcp/brem, 32x in the shrunk modes)
                o2p = st[bt]['o2p']
                o2rem = st[bt]['o2rem']
                hx = st[bt]['hx']
                for p in range(8):
                    for n in range(2):
                        ps = pp.tile([128, 384], f32, tag="big", bufs=4)
                        for c in range(2):
                            nc.tensor.matmul(ps[:], bcpv[:, c, :, p * 128:(p + 1) * 128],
                                             o2p[c][:, :, n * 384:(n + 1) * 384],
                                             start=(c == 0), stop=False, perf_mode=DR)
                        nc.tensor.matmul(ps[:], brem_all[:, p * 128:(p + 1) * 128],
                                         o2rem[:, n * 384:(n + 1) * 384],
                                         start=False, stop=True)
                        nc.vector.scalar_tensor_tensor(
                            hx[p][:, n * 384:(n + 1) * 384], ps[:], 1.0 / 1024.0,
                            hx[p][:, n * 384:(n + 1) * 384], op0=ALU.mult, op1=ALU.add)
                st[bt]['out1'] = hx

            def phase_E(bt):
                # LN2 -> h2 (normalized token-major bf16; affine folded into mw1/mb1)
                h2bf, _ = layernorm(st[bt]['out1'], p8, "h2", make_hx=False, bt=bt,
                                    nameprefix="h2")
                st[bt]['h2'] = h2bf

            def phase_F(bt, h):
                # MLP half: transpose h2 -> fp8 channel-major pairs, fp8 DoubleRow
                # 768->3072 gelu ->768, + res2, DMA out
                base = bt * NTOK
                h2bf = st[bt]['h2']
                out1 = st[bt]['out1']
                x2h = [px2.tile([128, 2, 512], fp8, tag="x2q", bufs=6,
                                name=f"x2h{bt}_{h}_{q}") for q in range(3)]
                for tcn in range(4):
                    p = h * 4 + tcn
                    for j in range(6):
                        pt = pp.tile([128, 128], bf16, tag="tpm")
                        nc.tensor.transpose(pt[:], h2bf[p][:, j * 128:(j + 1) * 128],
                                            ident_t[:])
                        nc.vector.tensor_copy(
                            x2h[j // 2][:, j % 2, tcn * 128:(tcn + 1) * 128], pt[:])
                hid = [phid.tile([128, 2, 512], fp8, tag="hid", bufs=24,
                                 name=f"hid{bt}_{h}_{qq}") for qq in range(12)]
                for fj in range(24):
                    ph = pp.tile([128, 512], f32, tag="big", bufs=4)
                    for q in range(3):
                        nc.tensor.matmul(ph[:], mw1_c(q, fj), x2h[q][:],
                                         start=(q == 0), stop=(q == 2),
                                         perf_mode=DR)
                    nc.scalar.activation(hid[fj // 2][:, fj % 2, :], ph[:],
                                         AF.Gelu, bias=mb1_c(fj), scale=1.0 / S1)
                for tcn in range(4):
                    p = h * 4 + tcn
                    ost = p8.tile([128, E], bf16, tag="xin", name=f"ost{bt}_{h}_{tcn}",
                                  bufs=16)
                    for n in range(2):
                        po = pp.tile([128, 384], f32, tag="big", bufs=4)
                        for qq in range(12):
                            nc.tensor.matmul(po[:],
                                             hid[qq][:, :, tcn * 128:(tcn + 1) * 128],
                                             mw2_c(qq, n),
                                             start=(qq == 0), stop=(qq == 11),
                                             perf_mode=DR)
                        nc.vector.scalar_tensor_tensor(
                            ost[:, n * 384:(n + 1) * 384], po[:], 1.0 / S2,
                            out1[p][:, n * 384:(n + 1) * 384],
                            op0=ALU.mult, op1=ALU.add)
                    if not mb2_zero:
                        nc.vector.tensor_add(ost[:], ost[:], mb2rep_t[:])
                    oeng = nc.sync if p % 2 == 0 else nc.scalar
                    oeng.dma_start(
                        out_d[base + p * 128: base + (p + 1) * 128, :], ost[:])

            phase_A(0)
            phase_A(1)
            fi_copy = phase_B(0)
            # deferred weight loads: don't let these race the startup burst
            # (x tiles + DFT matrices) on the HBM wire
            for dd_d, dd_t in ((bcp_d, bcp_all), (brem_d, brem_all),
                               (mw1_d, mw1_all), (mw2_d, mw2_all),
                               (mb1_d, mb1_all)):
                dd = nc.gpsimd.dma_start(dd_t[:], dd_d[:])
                add_dep_helper(dd.ins, fi_copy.ins,
                               reason="defer bulk weight DMA past fwd DFT")
            phase_B(1)
            phase_C(0)
            phase_C(1)
            phase_D(0)
            phase_E(0)
            phase_D(1)
            phase_E(1)
            phase_F(0, 0)
            phase_F(1, 0)
            phase_F(0, 1)
            phase_F(1, 1)

    nc.compile()
    return nc


LAST_EXEC_NS = None


def make_consts(w1, b1, w2, b2, ln1_g, ln1_b, ln2_g, ln2_b,
                mw1, mb1, mw2, mb2, ln1_trivial, mb2_zero):
    art, ait, brt, bit = _dft_matrices()

    # fold ln1_g into w1 (left-diag per block over the i axis)
    g_blocks = ln1_g.reshape(NB, BS)
    W1R = _pack_blockdiag(w1[0] * g_blocks[:, :, None])
    W1I = _pack_blockdiag(w1[1] * g_blocks[:, :, None])
    W2R = _pack_blockdiag(w2[0])
    W2I = _pack_blockdiag(w2[1])

    b1r = b1[0].reshape(E)
    b1i = b1[1].reshape(E)
    b2r = b2[0].reshape(E)
    b2i = b2[1].reshape(E)

    mw1f = mw1 * ln2_g[:, None]
    mb1f = (mb1 + ln2_b @ mw1).reshape(H4)

    def bf(a):
        return np.ascontiguousarray(a.astype(_BF16))

    def fp8_pairs(a, scale):
        """(2q*128, F) -> (128, q*2*F) k-pair-interleaved fp8 image for DoubleRow."""
        nq = a.shape[0] // 256
        img = (a * scale).reshape(nq, 2, 128, a.shape[1]) \
            .transpose(2, 0, 1, 3).reshape(128, 2 * nq * a.shape[1])
        return np.ascontiguousarray(img.astype(_FP8))

    # wmix image: (128, 6 mats * 6 blocks * 128), order w1r w1i w1in w2r w2i w2in
    # w1 carries the 1/32 that was removed from the fp8 DFT matrices
    mats = [W1R / 32.0, W1I / 32.0, -W1I / 32.0, W2R, W2I, -W2I]
    wmix = np.concatenate(
        [m.transpose(1, 0, 2).reshape(128, 6 * 128) for m in mats], axis=1)
    # bmix image: (128, 36): 6 vectors x 6 chunks; shrink biases carry the
    # 32x fp8-friendly scale on the shrunk modes (undone by 1/1024 after iDFT)
    bvecs = [b1r, b1i, 32.0 * (b2r - LAM), 32.0 * (-b2r - LAM),
             32.0 * (b2i - LAM), 32.0 * (-b2i - LAM)]
    bmix = np.concatenate([v.reshape(6, 128).T for v in bvecs], axis=1)

    # iDFT pair image: [128, c(2), k(2), 1024] slot0=brt, slot1=bit (32x scale)
    brt32 = brt * 32.0
    bit32 = bit * 32.0
    bcp = np.zeros((128, 2, 2, NTOK), np.float32)
    for c in range(2):
        bcp[:, c, 0, :] = brt32[c * 128:(c + 1) * 128]
        bcp[:, c, 1, :] = bit32[c * 128:(c + 1) * 128]
    brem = np.concatenate([brt32[256:288], bit32[256:288]], axis=0)

    consts = {
        "art": fp8_pairs(art, 32.0), "ait": fp8_pairs(ait, 32.0),
        "bcp": np.ascontiguousarray(bcp.reshape(128, 4 * NTOK).astype(_FP8)),
        "brem": bf(brem),
        "wmix": bf(wmix), "bmix": np.ascontiguousarray(bmix, np.float32),
        "mw1f": fp8_pairs(mw1f, S1),
        "mb1f": np.ascontiguousarray(mb1f.reshape(24, 128).T, np.float32),
        "mw2": fp8_pairs(mw2, S2),
        "ident": bf(np.eye(128, dtype=np.float32)),
    }
    if not ln1_trivial:
        consts["g1rep"] = np.tile(ln1_g[None, :], (128, 1)).astype(np.float32)
        consts["b1rep"] = np.tile(ln1_b[None, :], (128, 1)).astype(np.float32)
    if not mb2_zero:
        consts["mb2rep"] = np.tile(mb2[None, :], (128, 1)).astype(np.float32)
    return consts


def kernel(input, w1, b1, w2, b2, ln1_g, ln1_b, ln2_g, ln2_b, mw1, mb1, mw2, mb2):
    global LAST_EXEC_NS
    _install_trace_shim()
    import os
    from concourse.bass_utils import run_bass_kernel_spmd

    input = np.asarray(input, np.float32)
    w1 = np.asarray(w1, np.float32)
    b1 = np.asarray(b1, np.float32)
    w2 = np.asarray(w2, np.float32)
    b2 = np.asarray(b2, np.float32)
    ln1_g = np.asarray(ln1_g, np.float32)
    ln1_b = np.asarray(ln1_b, np.float32)
    ln2_g = np.asarray(ln2_g, np.float32)
    ln2_b = np.asarray(ln2_b, np.float32)
    mw1 = np.asarray(mw1, np.float32)
    mb1 = np.asarray(mb1, np.float32)
    mw2 = np.asarray(mw2, np.float32)
    mb2 = np.asarray(mb2, np.float32)

    ln1_trivial = bool(np.all(ln1_g == 1.0) and np.all(ln1_b == 0.0))
    mb2_zero = bool(np.all(mb2 == 0.0))

    key = (ln1_trivial, mb2_zero)
    if key not in _CACHE:
        _CACHE[key] = _build_program(ln1_trivial, mb2_zero)
    nc = _CACHE[key]

    consts = make_consts(w1, b1, w2, b2, ln1_g, ln1_b, ln2_g, ln2_b,
                         mw1, mb1, mw2, mb2, ln1_trivial, mb2_zero)

    xs = input.reshape(B * T, NTOK, E)
    in_maps = []
    for c in range(NCORES):
        shard = np.ascontiguousarray(
            xs[c * BT_PER_CORE:(c + 1) * BT_PER_CORE].reshape(TOK_CORE, E)
            .astype(_BF16))
        m = {"x": shard}
        m.update(consts)
        in_maps.append(m)

    trace = bool(os.environ.get("BASS_TRACE"))
    res = run_bass_kernel_spmd(nc, in_maps, core_ids=list(range(NCORES)),
                               trace=trace)
    LAST_EXEC_NS = res.exec_time_ns
    out = np.concatenate(
        [np.asarray(res.results[c]["out"]).astype(np.float32)
         .reshape(BT_PER_CORE, NTOK, E) for c in range(NCORES)], axis=0)
    return out.reshape(B, T, NTOK, E)

